# revision 1
# baseline (speedup 1.0000x reference)
"""Multi-head attention Bass kernel for Trainium2, 8-core SPMD.

Problem: B=2, S=2048, H=1024, 16 heads of 64 (torch-style MHA without
1/sqrt(d) scaling, key-padding mask, eval mode).

Sharding: core c handles batch b = c//4 and 4 heads (feature slice
256*(c%4) .. +256). Each core computes Q/K/V projections for its feature
slice over its batch, then attention for its 4 heads, producing
out[b, :, fslice]. Host concatenates.

Key-padding compaction: masked key positions contribute exactly
exp(-1e10) = 0 to softmax, so the host drops masked key/value rows and
pads to a multiple of 256 (typically 1280 of 2048 remain). Padding rows
get the -1e10 bias so they also contribute 0. Numerically identical to
the reference up to fp summation order.

Device-side layout: everything runs "transposed" (feature dim on
partitions); activations arrive pre-transposed from the host (layout
prep, like the weight transposes), so the device does no data
transposes except the tiny [65, q] output blocks:
  - Q^T, K^T [f, s]: scores S^T[kpos, q] = K^T.T @ Q^T (contraction
    d=64); the two heads of each 128-feature tile go to different PE
    row groups and run concurrently in the array
  - projections interleave with ACT-bound attention under a static
    8-bank PSUM plan; attention group g needs only K+Q[g], and V tiles
    are produced just ahead of their PV consumers
  - key-padding bias is per-kpos = per-partition -> folded into the
    exp() activation's bias operand
  - V kept [s, f] with an appended ones column per head, so the PV
    matmul gives out[0:64,:] = unnormalized out^T and out[64,:] = the
    softmax denominator
  - a small PE transpose of the [65, q] result gives [q, 65] where
    normalization (multiply by reciprocal of col 64) is a natural
    per-partition tensor_scalar op.
No max-subtraction in softmax: |scores| <~ 50 for randn-scale inputs,
exp fits fp32 comfortably (reference subtracts max; mathematically the
same ratio).

Matmuls run as float32r (4-byte storage, 1 PE cycle/row vs 4 for fp32,
tf32-like precision; measured output rel err ~8e-4 vs fp32 reference).
"""
import numpy as np

import concourse.bass as bass
import concourse.mybir as mybir
import concourse.tile as tile
from concourse.bass_utils import run_bass_kernel_spmd
from concourse.masks import make_identity

B, S, H = 2, 2048, 1024
NH, HD = 16, 64
N_CORES = 8
HPC = NH // (N_CORES // B)   # 4 heads per core
F = HPC * HD                 # 256 features per core
NEG = -10000000000.0

F32 = mybir.dt.float32
F32R = mybir.dt.float32r
MMDT = F32R


def _legalize_sync(nc, max_waits=1, max_updates=1):
    """This walrus build supports at most 1 sync wait / 1 sync update per
    instruction; split excess waits onto preceding same-engine NoOps."""
    n_upd = 0
    for f in nc.m.functions:
        for blk in f.blocks:
            out = []
            for inst in blk.instructions:
                si = getattr(inst, "sync_info", None)
                if si is not None and len(si.on_wait) > max_waits:
                    waits = list(si.on_wait)
                    for k, w in enumerate(waits[:-max_waits]):
                        out.append(mybir.InstNoOp(
                            name=f"{inst.name}-wsplit{k}",
                            sync_info=mybir.SyncInfo(on_wait=[w], on_update=[]),
                            bass_nofuse=True,
                            engine=inst.engine,
                        ))
                    inst.sync_info = mybir.SyncInfo(
                        on_wait=waits[-max_waits:], on_update=list(si.on_update))
                si = getattr(inst, "sync_info", None)
                if si is not None and len(si.on_update) > max_updates:
                    n_upd += 1
                out.append(inst)
            blk.instructions = out
    if n_upd:
        raise RuntimeError(f"{n_upd} instructions need >1 sync updates")


def _groups(total):
    """Split `total` positions into DMA/proj groups of <=512 (multiples
    of 256 so float32r matmuls stay at full rate)."""
    out = []
    pos = 0
    while pos < total:
        w = min(512, total - pos)
        out.append((pos, w))
        pos += w
    return out


def _emit(nc, tc, d, s_kv):
    from contextlib import ExitStack
    Exp = mybir.ActivationFunctionType.Exp
    Ident = mybir.ActivationFunctionType.Identity
    NQ = S // 512        # 4 query groups of 512
    NTQ = S // 128       # 16 query tiles of 128
    NTK = s_kv // 128    # key tiles of 128
    VW = F + HPC         # 260: V row-block width incl. ones columns

    with ExitStack() as ctx:
        const = ctx.enter_context(tc.tile_pool(name="const", bufs=1))
        ident32 = const.tile([128, 128], F32, tag="ident32", name="ident32")
        make_identity(nc, ident32)
        bqk_sb = const.tile([128, 4], F32, tag="bqk", name="bqk")
        nc.gpsimd.dma_start(bqk_sb[:, 0:2], d["bqr"])
        nc.gpsimd.dma_start(bqk_sb[:, 2:4], d["bkr"])
        mb_sb = const.tile([128, NTK], F32, tag="mb", name="mb")
        nc.gpsimd.dma_start(mb_sb[:], d["mbias"])
        bvb = const.tile([128, F], F32, tag="bvb", name="bvb")
        nc.gpsimd.dma_start(bvb[:], d["bvr"].to_broadcast((128, F)))

        qkv = ctx.enter_context(tc.tile_pool(name="qkv", bufs=1))
        # QT split per (m, 512-group) so attention on early q-groups can
        # start while later q-groups are still projecting
        QTt = [[qkv.tile([128, 512], MMDT, tag=f"qt{m}_{g}", name=f"qt{m}_{g}")
                for g in range(NQ)] for m in range(2)]
        KTm = [qkv.tile([128, s_kv], MMDT, tag=f"kt{m}", name=f"kt{m}")
               for m in range(2)]
        Vt = [qkv.tile([128, VW], MMDT, tag=f"v{t}", name=f"v{t}")
              for t in range(NTK)]
        outp = [qkv.tile([128, F], F32, tag=f"out{t}", name=f"out{t}")
                for t in range(NTQ)]
        for t in range(NTK):
            nc.gpsimd.memset(Vt[t][:].bitcast(mybir.dt.int32), 0x3F800000)

        wT_p = ctx.enter_context(tc.tile_pool(name="wT", bufs=1))
        xT_p = ctx.enter_context(tc.tile_pool(name="xT", bufs=3))
        es_p = ctx.enter_context(tc.tile_pool(name="expS", bufs=5))
        oT_p = ctx.enter_context(tc.tile_pool(name="oT", bufs=2))
        sm_p = ctx.enter_context(tc.tile_pool(name="sm", bufs=4))
        # 1-bank projection psum, alive through the whole kernel so the
        # q-projection overlaps ACT-bound attention (PSUM: 1+4+2+1 = 8)
        ps_qk = ctx.enter_context(
            tc.tile_pool(name="ps_qk", bufs=1, space="PSUM"))

        w_sb = {}
        for nm in ("wk", "wv", "wq"):
            w = wT_p.tile([128, 8 * F], MMDT, tag=nm, name=nm)
            nc.gpsimd.dma_start(
                w[:].rearrange("p (c f) -> p c f", c=8),
                d[nm + "T"].rearrange("(c p) f -> p c f", p=128))
            w_sb[nm] = w

        def load_xt(x_d, gpos, gw, slen):
            xT = xT_p.tile([128, 8 * 512], MMDT, tag="xT", name="xT")
            xTv = xT[:, 0:8 * gw].rearrange("p (c b) -> p c b", c=8)
            for c in range(8):
                nc.sync.dma_start(
                    xTv[:, c, :],
                    x_d[128 * c:128 * (c + 1), gpos:gpos + gw])
            return xTv

        # ---- K projection first (attention scores only need K) ----
        for gi, (gpos, gw) in enumerate(_groups(s_kv)):
            xTv = load_xt(d["xkT"], gpos, gw, s_kv)
            for m in range(2):
                pq = ps_qk.tile([128, 512], F32, tag="pq", name="pq")
                for c in range(8):
                    nc.tensor.matmul(
                        pq[:, 0:gw],
                        w_sb["wk"][:, 256 * c + 128 * m:
                                   256 * c + 128 * (m + 1)],
                        xTv[:, c, :],
                        start=(c == 0), stop=(c == 7))
                nc.vector.tensor_scalar(
                    KTm[m][:, gpos:gpos + gw], pq[:, 0:gw],
                    bqk_sb[:, 2 + m:3 + m], None, op0=mybir.AluOpType.add)


        def q_group(gi, gpos, gw):
            xTv = load_xt(d["xqT"], gpos, gw, S)
            for m in range(2):
                pq = ps_qk.tile([128, 512], F32, tag="pq", name="pq")
                for c in range(8):
                    nc.tensor.matmul(
                        pq[:],
                        w_sb["wq"][:, 256 * c + 128 * m:
                                   256 * c + 128 * (m + 1)],
                        xTv[:, c, :],
                        start=(c == 0), stop=(c == 7))
                nc.vector.tensor_scalar(
                    QTt[m][gi][:], pq[:], bqk_sb[:, m:m + 1], None,
                    op0=mybir.AluOpType.add)

        # Q group 0 right after K so attention can start; groups 1-3
        # after V, overlapping the ACT-bound attention phase.
        q_group(0, 0, 512)



        # ---- static PSUM plan: qk 1 + v 1 + scores 3 + acc 2 + tr 1 = 8
        # banks, all pools alive for the whole kernel so attention,
        # V-projection and q-projections interleave freely ----
        ps_vt = ctx.enter_context(
            tc.tile_pool(name="ps_vt", bufs=1, space="PSUM"))
        ps_s = ctx.enter_context(
            tc.tile_pool(name="ps_s", bufs=2, space="PSUM"))
        ps_o = ctx.enter_context(
            tc.tile_pool(name="ps_o", bufs=2, space="PSUM"))

        gv = _groups(s_kv)

        def v_group(vg):
            # V projection into per-kt tiles; attention PV matmuls chase
            # these tile by tile.
            gpos, gw = gv[vg]
            xTv = load_xt(d["xvT"], gpos, gw, s_kv)
            for j in range(gw // 128):
                pv = ps_vt.tile([128, F], F32, tag="pvt", name="pv")
                for c in range(8):
                    nc.tensor.matmul(
                        pv[:],
                        xTv[:, c, 128 * j:128 * (j + 1)],
                        w_sb["wv"][:, 256 * c:256 * (c + 1)],
                        start=(c == 0), stop=(c == 7))
                t = (gpos // 128) + j
                nc.vector.tensor_copy(
                    Vt[t][:].rearrange("p (h e) -> p h e", e=65)[:, :, 0:64],
                    pv[:].rearrange("p (h e) -> p h e", h=HPC))

        def c_group(g, v_after=()):
            for m in range(2):
                pv_lag = 2 if (v_after and m == 0) else 0
                # heads 2m (PE array rows 0-63) and 2m+1 (rows 64-127):
                # the two score matmuls go to different PE row groups
                # (tile_position auto-derived from base_partition) and
                # run concurrently in the array.
                acc0 = ps_o.tile([128, 512], F32, tag="acc", name="acc")
                acc1 = ps_o.tile([128, 512], F32, tag="acc", name="acc")
                h0, h1 = 2 * m, 2 * m + 1
                esq = []
                for kt in range(NTK + pv_lag):
                    # emit upcoming V projection groups just before their
                    # first PV consumer, so attention scores/exp start
                    # after only K+Q0 have loaded
                    if m == 0:
                        for vg, vstart in v_after:
                            if kt == vstart:
                                v_group(vg)
                    if kt < NTK:
                        ksl = slice(128 * kt, 128 * (kt + 1))
                        ps = ps_s.tile([128, 1024], F32, tag="ps", name="ps")
                        nc.tensor.matmul(
                            ps[:, 0:512], KTm[m][0:64, ksl],
                            QTt[m][g][0:64, :],
                            start=True, stop=True)
                        nc.tensor.matmul(
                            ps[:, 512:1024], KTm[m][64:128, ksl],
                            QTt[m][g][64:128, :],
                            start=True, stop=True)
                        es = es_p.tile([128, 1024], MMDT, tag="es", name="es")
                        nc.scalar.activation(
                            es[:], ps[:], Exp, bias=mb_sb[:, kt:kt + 1])
                        esq.append(es)
                    if kt >= pv_lag:
                        pk = kt - pv_lag
                        nc.tensor.matmul(
                            acc0[0:65, :], Vt[pk][:, 65 * h0:65 * (h0 + 1)],
                            esq[pk][:, 0:512],
                            start=(pk == 0), stop=(pk == NTK - 1))
                        nc.tensor.matmul(
                            acc1[0:65, :], Vt[pk][:, 65 * h1:65 * (h1 + 1)],
                            esq[pk][:, 512:1024],
                            start=(pk == 0), stop=(pk == NTK - 1))
                oT = oT_p.tile([128, 1024], F32, tag="oT", name="oT")
                nc.vector.tensor_copy(oT[0:65, 0:512], acc0[0:65, :])
                nc.vector.tensor_copy(oT[0:65, 512:1024], acc1[0:65, :])
                for hh in range(2):
                    h = 2 * m + hh
                    for j in range(4):
                        pt = ps_vt.tile([128, 65], F32, tag="pvt", name="ptt")
                        nc.tensor.transpose(
                            pt[:, 0:65],
                            oT[0:65, 512 * hh + 128 * j:512 * hh + 128 * (j + 1)],
                            ident32[0:65, 0:65])
                        rc = sm_p.tile([128, 1], F32, tag="rc", name="rc")
                        nc.vector.reciprocal(rc[:], pt[:, 64:65])
                        tmp = sm_p.tile([128, 64], F32, tag="tmp", name="tmp")
                        nc.vector.tensor_scalar_mul(tmp[:], pt[:, 0:64], rc[:])
                        nc.vector.tensor_add(
                            outp[4 * g + j][:, 64 * h:64 * (h + 1)],
                            tmp[:], bvb[:, 64 * h:64 * (h + 1)])
            for j in range(4):
                nc.sync.dma_start(
                    d["out"][512 * g + 128 * j:512 * g + 128 * (j + 1), :],
                    outp[4 * g + j][:])

        # attention group 0 first (needs only K + Q0 + the first V
        # group); later V groups are emitted inside C0's kt loop just
        # ahead of their consumers, and the other q-projections fill PE
        # gaps under the ACT-bound attention.
        gq = _groups(S)
        c_group(0, v_after=[(0, 2), (1, 4), (2, 7)][:len(gv)])
        for g in range(1, NQ):
            q_group(g, *gq[g])
            c_group(g)


_NC_CACHE = {}


def _build(s_kv):
    if s_kv in _NC_CACHE:
        return _NC_CACHE[s_kv]
    nc = bass.Bass(trn_type="TRN2", target_bir_lowering=False, debug=False)
    d = {
        "xqT": nc.dram_tensor("xqT", [H, S], MMDT, kind="ExternalInput").ap(),
        "xkT": nc.dram_tensor("xkT", [H, s_kv], MMDT, kind="ExternalInput").ap(),
        "xvT": nc.dram_tensor("xvT", [H, s_kv], MMDT, kind="ExternalInput").ap(),
        "wqT": nc.dram_tensor("wqT", [H, F], MMDT, kind="ExternalInput").ap(),
        "wkT": nc.dram_tensor("wkT", [H, F], MMDT, kind="ExternalInput").ap(),
        "wvT": nc.dram_tensor("wvT", [H, F], MMDT, kind="ExternalInput").ap(),
        "bqr": nc.dram_tensor("bqr", [128, 2], F32, kind="ExternalInput").ap(),
        "bkr": nc.dram_tensor("bkr", [128, 2], F32, kind="ExternalInput").ap(),
        "bvr": nc.dram_tensor("bvr", [1, F], F32, kind="ExternalInput").ap(),
        "mbias": nc.dram_tensor("mbias", [128, s_kv // 128], F32,
                                kind="ExternalInput").ap(),
        "out": nc.dram_tensor("out", [S, F], F32, kind="ExternalOutput").ap(),
    }
    with tile.TileContext(nc) as tc:
        _emit(nc, tc, d, s_kv)
    _legalize_sync(nc)
    _NC_CACHE[s_kv] = nc
    return nc


def plan_kv(mask):
    """Per-batch compaction plan: indices of valid key positions and the
    padded kv length shared across batches."""
    mask = np.asarray(mask)
    idxs = [np.nonzero(mask[b])[0] for b in range(B)]
    nmax = max((len(i) for i in idxs), default=1)
    s_kv = min(S, max(256, -(-nmax // 256) * 256))
    return idxs, s_kv


def make_in_maps(query, key, value, mask, Wq, bq, Wk, bk, Wv, bv,
                 idxs=None, s_kv=None):
    if idxs is None:
        idxs, s_kv = plan_kv(mask)
    query, key, value = (np.asarray(a, np.float32) for a in (query, key, value))
    Wq, Wk, Wv = (np.asarray(a, np.float32) for a in (Wq, Wk, Wv))
    bq, bk, bv = (np.asarray(a, np.float32) for a in (bq, bk, bv))
    in_maps = []
    qc, kc, vc, mbc = {}, {}, {}, {}
    for b in range(B):
        idx = idxs[b]
        qc[b] = np.ascontiguousarray(query[b].T)
        kcb = np.zeros((H, s_kv), np.float32)
        kcb[:, :len(idx)] = key[b][idx].T
        vcb = np.zeros((H, s_kv), np.float32)
        vcb[:, :len(idx)] = value[b][idx].T
        mb = np.full(s_kv, NEG, np.float32)
        mb[:len(idx)] = 0.0
        kc[b], vc[b] = kcb, vcb
        mbc[b] = np.ascontiguousarray(mb.reshape(s_kv // 128, 128).T)
    for c in range(N_CORES):
        b = c // (N_CORES // B)
        fs = F * (c % (N_CORES // B))
        in_maps.append({
            "xqT": qc[b],
            "xkT": kc[b],
            "xvT": vc[b],
            "wqT": np.ascontiguousarray(Wq[fs:fs + F].T),
            "wkT": np.ascontiguousarray(Wk[fs:fs + F].T),
            "wvT": np.ascontiguousarray(Wv[fs:fs + F].T),
            "bqr": np.ascontiguousarray(bq[fs:fs + F].reshape(2, 128).T),
            "bkr": np.ascontiguousarray(bk[fs:fs + F].reshape(2, 128).T),
            "bvr": np.ascontiguousarray(bv[fs:fs + F].reshape(1, F)),
            "mbias": mbc[b],
        })
    return in_maps


def assemble(results):
    out = np.empty((B, S, H), np.float32)
    for c in range(N_CORES):
        b = c // (N_CORES // B)
        fs = F * (c % (N_CORES // B))
        out[b, :, fs:fs + F] = results[c]["out"]
    return out


def kernel(query, key, value, mask, Wq, bq, Wk, bk, Wv, bv, _trace=False):
    idxs, s_kv = plan_kv(mask)
    nc = _build(s_kv)
    in_maps = make_in_maps(query, key, value, mask, Wq, bq, Wk, bk, Wv, bv,
                           idxs, s_kv)
    res = run_bass_kernel_spmd(nc, in_maps, core_ids=list(range(N_CORES)),
                               trace=_trace)
    out = assemble(res.results)
    if _trace:
        return out, res
    return out

